# revision 10
# baseline (speedup 1.0000x reference)
"""Trainium2 Bass kernel for nn_AttnBFAN (batched attention w/ focal re-norm).

Data-parallel over the batch dim: 128 batches sharded 16-per-core across 8
NeuronCores. Per batch (Q=128, C=1024, D=1024):
    attn = leaky_relu(context @ query^T, 0.1)          (C, Q)
    attn = attn / (||attn||_2 over q)                  l2norm per (b, c)
    p    = softmax(20 * attn^T, axis=c)                (Q, C)
    t    = (p > mean_c p) * p ; re_attn = t / sum_c t
    wcontext = re_attn @ context                       (Q, D)
returns (query, wcontext, re_attn).

v9: ridge-balance PE vs DMA; lean ACT/DVE chain.
 - Host pre-transposes d-chunks 0..3 of context (extra `contextT` input):
   those 4 ctx^T chunks arrive by DMA (+1MB/batch on the underused DMA)
   and the PE only transposes chunks 4..7 (32 instead of 64 transposes
   per batch). The PSUM->SBUF evictions for the remaining chunks go to
   DVE (mid) / ACT+Pool (late) where each has queue slack.
 - Prelu runs on DVE as max(0.1x, x); wcontext eviction runs on Pool
   (gpsimd) with the 1/sum_c(t) renorm folded in, bf16 out.
 - re_attn is not materialized on device: the raw focal weights t (bf16)
   and the per-(b,q) reciprocal sums rinv ship to HBM and the host
   multiplies them out. Outputs are bf16 (upcast on host), halving
   store traffic.
 - PE stream per iter b: T6,7(b+1) | ones(b) | bmm1(b+1) | T4,5(b+2) |
   t^T(b) | bmm2(b). The chain tail (Ln->Exp->u->exp->focal) hides
   behind bmm1(b+1)+T45; PSUM: a 2 banks, w/S 2, tT 1, transpose ring 3.
"""

import os
import numpy as np
import ml_dtypes

import concourse.bacc as bacc
import concourse.mybir as mybir
import concourse.tile as tile
from concourse.bass_utils import run_bass_kernel_spmd
from concourse.masks import make_identity
from concourse.hw_specs import get_activation_tables

F32 = mybir.dt.float32
F32R = mybir.dt.float32r
BF16 = mybir.dt.bfloat16
AX = mybir.AxisListType
ALU = mybir.AluOpType
ACTF = mybir.ActivationFunctionType

NCORES = 8
NB = 128          # total batches
BPC = NB // NCORES  # batches per core
Q = 128
C = 1024
D = 1024
SMOOTH = 20.0
KT = 4            # d-chunks of ctx^T supplied pre-transposed from HBM

_CACHE = {}
STAGES = []  # (label, first_instruction_id) build-time markers for tracing


def _build():
    nc = bacc.Bacc("TRN2", target_bir_lowering=False, debug=False,
                   num_devices=NCORES, name="attn_bfan")

    def mark(label):
        STAGES.append((label, int(nc.get_next_instruction_name().split("-")[1])))

    # query pre-transposed+tiled on host: [b, p(=d%128), jd, q] bf16
    q_in = nc.dram_tensor("query", [BPC, 128, 8, Q], BF16, kind="ExternalInput")
    # context pre-tiled on host: [b, p(=c%128), jc, d] bf16
    c_in = nc.dram_tensor("context", [BPC, 128, 8, D], BF16, kind="ExternalInput")
    # context^T d-chunks 0..KT-1 pre-transposed on host: [b, p(=d%128), jd, c]
    ct_in = nc.dram_tensor("contextT", [BPC, 128, KT, C], BF16,
                           kind="ExternalInput")
    t_out = nc.dram_tensor("t_out", [BPC, Q, C], BF16, kind="ExternalOutput")
    wc_out = nc.dram_tensor("wcontext", [BPC, Q, D], BF16, kind="ExternalOutput")
    rv_out = nc.dram_tensor("rv_out", [Q, BPC], F32, kind="ExternalOutput")

    with tile.TileContext(nc) as tc:
        with (
            tc.tile_pool(name="singles", bufs=1) as singles,
            tc.tile_pool(name="ctxp", bufs=4) as ctxp,
            tc.tile_pool(name="ctxtp", bufs=2) as ctxtp,
            tc.tile_pool(name="qTp", bufs=3) as qTp,
            tc.tile_pool(name="tTp", bufs=2) as tTp,
            tc.tile_pool(name="work", bufs=2) as work,
            tc.tile_pool(name="w1", bufs=1) as w1,
            tc.tile_pool(name="tpool", bufs=2) as tpool,
            tc.tile_pool(name="stat", bufs=2) as stat,
            tc.tile_pool(name="ps_a", bufs=1, space="PSUM") as ps_a,
            tc.tile_pool(name="ps_w", bufs=1, space="PSUM") as ps_w,
            tc.tile_pool(name="ps_f", bufs=1, space="PSUM") as ps_f,
            tc.tile_pool(name="ps_tp", bufs=3, space="PSUM") as ps_tp,
        ):
            tab_names = list(get_activation_tables("gen3").keys())
            nc.scalar.add_instruction(mybir.InstLoadActFuncSet(
                name=nc.get_next_instruction_name(),
                act_func_set_id=tab_names.index("natural_log_exp_and_others"),
                ins=[], outs=[]))
            ident = singles.tile([128, 128], F32, tag="ident")
            make_identity(nc, ident[:])
            identb = singles.tile([128, 128], BF16, tag="identb")
            nc.vector.tensor_copy(identb[:], ident[:])
            ones_f = singles.tile([128, 128], F32, tag="ones_f")
            nc.vector.memset(ones_f[:], 1.0)
            ones_r = singles.tile([128, 128], F32R, tag="ones_r")
            nc.vector.tensor_copy(ones_r[:], ones_f[:])
            ln20 = singles.tile([128, 1], F32, tag="ln20")
            nc.vector.memset(ln20[:], float(np.log(SMOOTH)))
            invC = singles.tile([128, 1], F32, tag="invC")
            nc.vector.memset(invC[:], 1.0 / C)
            rv_all = singles.tile([128, BPC], F32, tag="rv_all")

            ctx_t = [None] * (BPC + 3)   # plain ctx bf16 [128, 8jc, 1024d]
            ctxT_t = [None] * (BPC + 3)  # ctx^T bf16 [128, 8jd, 1024c]
            qT_t = [None] * (BPC + 3)    # q^T bf16 [128, 8jd, 128q]

            def load_batch(b):
                ctx = ctxp.tile([128, 8, D], BF16, tag="ctx", name="ctx")
                nc.gpsimd.dma_start(out=ctx[:, 0:6, :], in_=c_in[b][:, 0:6, :])
                nc.sync.dma_start(out=ctx[:, 6:8, :], in_=c_in[b][:, 6:8, :])
                ctx_t[b] = ctx
                qT = qTp.tile([128, 8, Q], BF16, tag="qT", name="qT")
                nc.sync.dma_start(out=qT[:], in_=q_in[b])
                qT_t[b] = qT

            def load_ctxT(b):
                # ctx^T chunks 0..KT-1 straight from HBM (host-transposed)
                ctxT = ctxtp.tile([128, 8, C], BF16, tag="ctxT", name="ctxT")
                nc.scalar.dma_start(out=ctxT[:, 0:KT, :], in_=ct_in[b])
                ctxT_t[b] = ctxT

            def transpose_jd_pe(b, jd, pool=None):
                # PE-transpose ctx d-chunk jd into a 1-bank PSUM tile
                ctx = ctx_t[b]
                tp = (pool or ps_tp).tile([128, 8, 128], BF16, tag="tp",
                                          name="tp")
                for jc in range(8):
                    nc.tensor.transpose(
                        tp[:, jc, :],
                        ctx[:, jc, jd * 128:(jd + 1) * 128], identb[:])
                return tp

            def copy_jd(b, jd, tp, copy_eng):
                # evict one transposed d-chunk: 1024-elem bf16 PSUM->SBUF copy
                src = tp[:].rearrange("p a b -> p (a b)")
                dst = ctxT_t[b][:, jd, :]
                if copy_eng == "act":
                    nc.scalar.copy(dst, src)
                elif copy_eng == "pool":
                    nc.gpsimd.tensor_copy(dst, src)
                else:
                    nc.vector.tensor_copy(dst, src)

            def transpose_jd(b, jd, copy_eng, pool=None):
                copy_jd(b, jd, transpose_jd_pe(b, jd, pool), copy_eng)

            def bmm1(b):
                # attn^T (q, c) accumulated over 8 d-chunks -> a0/a1
                a0 = ps_a.tile([128, 512], F32, tag="a0", name="a0")
                a1 = ps_a.tile([128, 512], F32, tag="a1", name="a1")
                qT = qT_t[b]
                ctxT = ctxT_t[b]
                for jd in range(8):
                    st, sp = jd == 0, jd == 7
                    nc.tensor.matmul(a0[:], qT[:, jd, :], ctxT[:, jd, 0:512],
                                     start=st, stop=sp)
                    nc.tensor.matmul(a1[:], qT[:, jd, :], ctxT[:, jd, 512:1024],
                                     start=st, stop=sp)
                return a0, a1

            # ---- prologue: batch 0 fully staged, 1 mostly-transposed.
            # ctx(0) split across all three queues to cut first-batch latency
            ctx0 = ctxp.tile([128, 8, D], BF16, tag="ctx", name="ctx")
            nc.gpsimd.dma_start(out=ctx0[:, 0:3, :], in_=c_in[0][:, 0:3, :])
            nc.sync.dma_start(out=ctx0[:, 3:6, :], in_=c_in[0][:, 3:6, :])
            nc.scalar.dma_start(out=ctx0[:, 6:8, :], in_=c_in[0][:, 6:8, :])
            ctx_t[0] = ctx0
            qT0 = qTp.tile([128, 8, Q], BF16, tag="qT", name="qT")
            nc.scalar.dma_start(out=qT0[:], in_=q_in[0])
            qT_t[0] = qT0
            load_ctxT(0)
            load_ctxT(1)
            load_batch(1)
            load_batch(2)
            # bmm1(0) needs only batch 0's transposes; batch 1's {4,5} follow
            for jd, eng in zip(range(KT, 8), ("vec", "act", "vec", "act")):
                transpose_jd(0, jd, eng)
            a_cur = bmm1(0)
            transpose_jd(1, 4, "act")
            transpose_jd(1, 5, "vec")
            del jd, eng

            h0, h1 = slice(0, 512), slice(512, 1024)

            def emit_chain(b, a0, a1, spool, stags, mid=None):
                # prelu(DVE) -> sq -> [mid hook] -> ones(S into spool) ->
                # ln/exp -> u -> exp+accum -> focal. Returns (t, ts0, ts1).
                mark(f'i{b}_prelu')
                attn = work.tile([128, C], F32, tag="attn", name="attn")
                nc.scalar.activation(attn[:, h0], a0[:], ACTF.Prelu,
                                     bias=0.0, scale=1.0, alpha=0.1)
                nc.scalar.activation(attn[:, h1], a1[:], ACTF.Prelu,
                                     bias=0.0, scale=1.0, alpha=0.1)
                sq = w1.tile([128, C], F32R, tag="w1a", name="sq")
                nc.vector.tensor_mul(sq[:, h0], attn[:, h0], attn[:, h0])
                nc.vector.tensor_mul(sq[:, h1], attn[:, h1], attn[:, h1])
                if mid is not None:
                    mid()
                # l2 norm: ones-matmul sums over q and broadcasts
                mark(f'i{b}_ones')
                s0 = spool.tile([128, 512], F32, tag=stags[0], name="s0")
                s1 = spool.tile([128, 512], F32, tag=stags[1], name="s1")
                nc.tensor.matmul(s0[:], ones_r[:], sq[:, h0], start=True, stop=True)
                nc.tensor.matmul(s1[:], ones_r[:], sq[:, h1], start=True, stop=True)
                # 20/sqrt(S) = exp(-0.5*ln(S) + ln 20)
                mark(f'i{b}_ln')
                lnS = w1.tile([128, C], F32, tag="w1b", name="lnS")
                nc.scalar.activation(lnS[:, h0], s0[:], ACTF.Ln)
                nc.scalar.activation(lnS[:, h1], s1[:], ACTF.Ln)
                rn20 = w1.tile([128, C], F32, tag="w1c", name="rn20")
                u = w1.tile([128, C], F32, tag="w1a", name="u")
                pu = work.tile([128, C], F32, tag="pu", name="pu")
                rs0 = stat.tile([128, 1], F32, tag="rs0", name="rs0")
                rs1 = stat.tile([128, 1], F32, tag="rs1", name="rs1")
                nc.scalar.activation(rn20[:, h0], lnS[:, h0], ACTF.Exp,
                                     bias=ln20[:], scale=-0.5)
                nc.scalar.activation(rn20[:, h1], lnS[:, h1], ACTF.Exp,
                                     bias=ln20[:], scale=-0.5)
                nc.vector.tensor_mul(u[:, h0], attn[:, h0], rn20[:, h0])
                nc.vector.tensor_mul(u[:, h1], attn[:, h1], rn20[:, h1])
                nc.scalar.activation(pu[:, h0], u[:, h0], ACTF.Exp,
                                     bias=0.0, scale=1.0, accum_out=rs0[:])
                nc.scalar.activation(pu[:, h1], u[:, h1], ACTF.Exp,
                                     bias=0.0, scale=1.0, accum_out=rs1[:])
                # thr = (rs0 + rs1) / C in one DVE op
                thr = stat.tile([128, 1], F32, tag="thr", name="thr")
                nc.vector.scalar_tensor_tensor(
                    out=thr[:], in0=rs0[:], scalar=rs1[:], in1=invC[:],
                    op0=ALU.add, op1=ALU.mult)
                # focal: t = (pu > thr) * pu (bf16), half-split so the t^T
                # transposes pipeline behind stt-h0
                mark(f'i{b}_focal')
                t = tpool.tile([128, C], BF16, tag="t", name="t")
                ts0 = stat.tile([128, 1], F32, tag="ts0", name="ts0")
                ts1 = stat.tile([128, 1], F32, tag="ts1", name="ts1")
                nc.vector.scalar_tensor_tensor(
                    out=t[:, h0], in0=pu[:, h0], scalar=thr[:], in1=pu[:, h0],
                    op0=ALU.is_gt, op1=ALU.mult, accum_out=ts0[:])
                nc.vector.scalar_tensor_tensor(
                    out=t[:, h1], in0=pu[:, h1], scalar=thr[:], in1=pu[:, h1],
                    op0=ALU.is_gt, op1=ALU.mult, accum_out=ts1[:])
                return t, ts0, ts1

            def emit_tail(b, t, ts0, ts1, late=None):
                # t^T transposes (half-pipelined), rinv, late ctx^T copies,
                # t store, bmm2, wc eviction (Pool) scaled by rinv
                mark(f'i{b}_tT')
                tT = tTp.tile([128, 8, Q], BF16, tag="tT", name="tT")
                tpf = ps_f.tile([128, 8, 128], BF16, tag="tpf", name="tpf")
                for jc in range(8):
                    nc.tensor.transpose(
                        tpf[:, jc, :],
                        t[:, jc * 128:(jc + 1) * 128], identb[:])
                    if jc == 3:
                        nc.vector.tensor_copy(
                            tT[:, 0:4, :].rearrange("p a b -> p (a b)"),
                            tpf[:, 0:4, :].rearrange("p a b -> p (a b)"))
                nc.vector.tensor_copy(
                    tT[:, 4:8, :].rearrange("p a b -> p (a b)"),
                    tpf[:, 4:8, :].rearrange("p a b -> p (a b)"))
                ts = stat.tile([128, 1], F32, tag="ts", name="ts")
                nc.vector.tensor_add(ts[:], ts0[:], ts1[:])
                rv = rv_all[:, b:b + 1]
                nc.vector.reciprocal(rv, ts[:])
                if late is not None:
                    late()
                # raw focal weights out; host multiplies by rinv
                if b == BPC - 1:
                    nc.scalar.dma_start(out=t_out[b][:, 0:512], in_=t[:, 0:512])
                    nc.gpsimd.dma_start(out=t_out[b][:, 512:1024],
                                        in_=t[:, 512:1024])
                else:
                    nc.scalar.dma_start(out=t_out[b], in_=t[:])
                mark(f'i{b}_bmm2')
                ctx = ctx_t[b]
                w0 = ps_w.tile([128, 512], F32, tag="w0", name="w0")
                w2 = ps_w.tile([128, 512], F32, tag="w2", name="w2")
                for jc in range(8):
                    st, sp = jc == 0, jc == 7
                    nc.tensor.matmul(w0[:], tT[:, jc, :], ctx[:, jc, 0:512],
                                     start=st, stop=sp)
                    nc.tensor.matmul(w2[:], tT[:, jc, :], ctx[:, jc, 512:1024],
                                     start=st, stop=sp)
                mark(f'i{b}_wc')
                wc = work.tile([128, D], BF16, tag="wc", name="wc")
                nc.scalar.activation(wc[:, h0], w0[:], ACTF.Copy,
                                     bias=0.0, scale=rv)
                _wc1 = nc.vector.tensor_scalar(wc[:, h1], w2[:], rv, None,
                                               ALU.mult)
                if b == BPC - 1:
                    # last batch: fan the store across the now-idle queues
                    wcd = wc_out[b].rearrange("q (g e) -> q g e", g=4)
                    wcs = wc[:].rearrange("q (g e) -> q g e", g=4)
                    nc.sync.dma_start(out=wcd[:, 0:2, :], in_=wcs[:, 0:2, :])
                    nc.gpsimd.dma_start(out=wcd[:, 2:3, :], in_=wcs[:, 2:3, :])
                    nc.scalar.dma_start(out=wcd[:, 3:4, :], in_=wcs[:, 3:4, :])
                else:
                    nc.sync.dma_start(out=wc_out[b], in_=wc[:])

            for b in range(BPC - 1):  # batch BPC-1 is merged into BPC-2
                mark(f'iter{b}')
                if b + 3 < BPC:
                    load_batch(b + 3)
                if b + 2 < BPC:
                    load_ctxT(b + 2)

                def mid_tp(b=b):
                    # finish next batch's ctx^T (jd 6,7) in the pre-Ln slack
                    mark(f'i{b}_tp67')
                    transpose_jd(b + 1, 6, "vec")
                    transpose_jd(b + 1, 7, "vec")

                t, ts0c, ts1c = emit_chain(b, *a_cur, ps_w, ("w0", "w2"),
                                           mid=mid_tp)

                # PE: bmm1(b+1) + first transposes of b+2 fill the chain tail
                mark(f'i{b}_bmm1n')
                a_cur = bmm1(b + 1)
                tp_late = []
                if b + 2 < BPC:
                    mark(f'i{b}_tp45')
                    tp_late.append((4, "act", transpose_jd_pe(b + 2, 4)))
                    tp_late.append((5, "vec", transpose_jd_pe(b + 2, 5)))

                def late(b=b, tp_late=tp_late):
                    for jd, eng, tp in tp_late:
                        copy_jd(b + 2, jd, tp, eng)

                if b == BPC - 2:
                    # epilogue: interleave the last batch's chain (its l2 sums
                    # use the freed a-banks); batch b's tail fills its latency
                    na0, na1 = a_cur
                    t15, ts015, ts115 = emit_chain(
                        BPC - 1, na0, na1, ps_a, ("a0", "a1"),
                        mid=lambda: emit_tail(b, t, ts0c, ts1c, late=late))
                    emit_tail(BPC - 1, t15, ts015, ts115)
                else:
                    emit_tail(b, t, ts0c, ts1c, late=late)
                ctx_t[b] = None
                ctxT_t[b] = None
                qT_t[b] = None

            nc.sync.dma_start(out=rv_out[:, :], in_=rv_all[:])

    nc.compile()
    return nc


def kernel(query: np.ndarray, context: np.ndarray):
    query = np.ascontiguousarray(query, dtype=np.float32)
    context = np.ascontiguousarray(context, dtype=np.float32)
    assert query.shape == (NB, Q, D) and context.shape == (NB, C, D)

    if "nc" not in _CACHE:
        _CACHE["nc"] = _build()
    nc = _CACHE["nc"]

    bf16 = ml_dtypes.bfloat16
    # qT host prep: (B, Q, D) -> [b, p, jd, q] where d = jd*128 + p
    qT = np.ascontiguousarray(
        query.transpose(0, 2, 1).reshape(NB, 8, 128, Q).transpose(0, 2, 1, 3)
    ).astype(bf16)
    # context: (B, C, D) -> [b, p, jc, d] with c = jc*128 + p
    ctx_bf = np.ascontiguousarray(
        context.reshape(NB, 8, 128, D).transpose(0, 2, 1, 3)
    ).astype(bf16)
    # context^T chunks 0..KT-1: [b, p, jd, c] with d = jd*128 + p
    ctxT = np.ascontiguousarray(
        context[:, :, 0:KT * 128].transpose(0, 2, 1)
        .reshape(NB, KT, 128, C).transpose(0, 2, 1, 3)
    ).astype(bf16)

    in_maps = []
    for k in range(NCORES):
        sl = slice(k * BPC, (k + 1) * BPC)
        in_maps.append({"query": qT[sl], "context": ctx_bf[sl],
                        "contextT": ctxT[sl]})

    trace = os.environ.get("KERNEL_TRACE", "0") == "1"
    res = run_bass_kernel_spmd(nc, in_maps, core_ids=list(range(NCORES)),
                               trace=trace)
    _CACHE["last_res"] = res

    # host renorm: re = t * rinv (broadcast over c), wc upcast
    re_parts = []
    wc_parts = []
    for r in res.results:
        t = np.asarray(r["t_out"]).astype(np.float32)        # [BPC, Q, C]
        rv = np.asarray(r["rv_out"]).astype(np.float32)      # [Q, BPC]
        wc = np.asarray(r["wcontext"]).astype(np.float32)    # [BPC, Q, D]
        re_parts.append(t * rv.T[:, :, None])
        wc_parts.append(wc)
    re_attn = np.concatenate(re_parts, axis=0)
    wcontext = np.concatenate(wc_parts, axis=0)
    return query, wcontext, re_attn


# revision 19
# speedup vs baseline: 1.5567x; 1.5567x over previous
"""Trainium2 Bass kernel for nn_AttnBFAN (batched attention w/ focal re-norm).

Data-parallel over the batch dim: 128 batches sharded 16-per-core across 8
NeuronCores. Per batch (Q=128, C=1024, D=1024):
    attn = leaky_relu(context @ query^T, 0.1)          (C, Q)
    attn = attn / (||attn||_2 over q)                  l2norm per (b, c)
    p    = softmax(20 * attn^T, axis=c)                (Q, C)
    t    = (p > mean_c p) * p ; re_attn = t / sum_c t
    wcontext = re_attn @ context                       (Q, D)
returns (query, wcontext, re_attn).

v10: 3-phase software pipeline so no PE op ever waits on the softmax
chain. Each batch's work is spread over ~2.5 iterations:
    iter b-1:  prelu(b), sq(b), ones(b)          [head]
    iter b:    Ln, Exp, u, pu, thr, focal(b)     [chain tail]
    iter b+1:  t^T(b), bmm2(b), wc(b)            [tail]
so every PE instruction in iter b (tT(b-1), bmm2(b-1), ones(b+1),
T4567(b+3), bmm1(b+2)) depends only on work finished in earlier
iterations -- the PE streams continuously and stays at the 2.4 GHz
p-state. Other v10 features:
 - Host pre-transposes d-chunks 0..KT-1 of context (extra `contextT`
   input); the PE transposes only the rest (DMA and PE balanced at the
   ridge).
 - The ctx^T/t^T PSUM tiles share one 2-bank PSUM ring; a/S/w pairs
   take the other six banks.
 - u = attn*rn20 runs on Pool (gpsimd); everything else element-wise is
   split ACT/DVE.
 - re_attn is not materialized: raw focal t (bf16) + per-(b,q) rinv
   ship out and the host multiplies. wcontext ships bf16.
"""

import os
import numpy as np
import ml_dtypes

import concourse.bacc as bacc
import concourse.mybir as mybir
import concourse.tile as tile
from concourse.bass_utils import run_bass_kernel_spmd
from concourse.masks import make_identity
from concourse.hw_specs import get_activation_tables

F32 = mybir.dt.float32
F32R = mybir.dt.float32r
BF16 = mybir.dt.bfloat16
AX = mybir.AxisListType
ALU = mybir.AluOpType
ACTF = mybir.ActivationFunctionType

NCORES = 8
NB = 128          # total batches
BPC = NB // NCORES  # batches per core
Q = 128
C = 1024
D = 1024
SMOOTH = 20.0
KT = 4            # d-chunks of ctx^T supplied pre-transposed from HBM

_CACHE = {}
STAGES = []  # (label, first_instruction_id) build-time markers for tracing


def _build():
    nc = bacc.Bacc("TRN2", target_bir_lowering=False, debug=False,
                   num_devices=NCORES, name="attn_bfan")

    def mark(label):
        STAGES.append((label, int(nc.get_next_instruction_name().split("-")[1])))

    # query pre-transposed+tiled on host: [b, p(=d%128), jd, q] bf16
    q_in = nc.dram_tensor("query", [BPC, 128, 8, Q], BF16, kind="ExternalInput")
    # context pre-tiled on host: [b, p(=c%128), jc, d] bf16
    c_in = nc.dram_tensor("context", [BPC, 128, 8, D], BF16, kind="ExternalInput")
    # context^T d-chunks 0..KT-1 pre-transposed on host: [b, p(=d%128), jd, c]
    ct_in = nc.dram_tensor("contextT", [BPC, 128, KT, C], BF16,
                           kind="ExternalInput")
    # prologue batches 0..2 get the remaining chunks from the host too
    ctp_in = nc.dram_tensor("contextTpro", [3, 128, 8 - KT, C], BF16,
                            kind="ExternalInput")
    t_out = nc.dram_tensor("t_out", [BPC, Q, C], BF16, kind="ExternalOutput")
    wc_out = nc.dram_tensor("wcontext", [BPC, Q, D], BF16, kind="ExternalOutput")
    rv_out = nc.dram_tensor("rv_out", [Q, BPC], F32, kind="ExternalOutput")

    with tile.TileContext(nc) as tc:
        with (
            tc.tile_pool(name="singles", bufs=1) as singles,
            tc.tile_pool(name="ctxp", bufs=6) as ctxp,
            tc.tile_pool(name="ctxtp", bufs=2) as ctxtp,
            tc.tile_pool(name="qTp", bufs=3) as qTp,
            tc.tile_pool(name="tTp", bufs=2) as tTp,
            tc.tile_pool(name="work", bufs=2) as work,
            tc.tile_pool(name="w1", bufs=1) as w1,
            tc.tile_pool(name="tpool", bufs=2) as tpool,
            tc.tile_pool(name="stat", bufs=2) as stat,
            tc.tile_pool(name="ps_a", bufs=1, space="PSUM") as ps_a,
            tc.tile_pool(name="ps_s", bufs=1, space="PSUM") as ps_s,
            tc.tile_pool(name="ps_w", bufs=1, space="PSUM") as ps_w,
            tc.tile_pool(name="ps_tp", bufs=2, space="PSUM") as ps_tp,
        ):
            tab_names = list(get_activation_tables("gen3").keys())
            nc.scalar.add_instruction(mybir.InstLoadActFuncSet(
                name=nc.get_next_instruction_name(),
                act_func_set_id=tab_names.index("natural_log_exp_and_others"),
                ins=[], outs=[]))
            ident = singles.tile([128, 128], F32, tag="ident")
            make_identity(nc, ident[:])
            identb = singles.tile([128, 128], BF16, tag="identb")
            nc.vector.tensor_copy(identb[:], ident[:])
            ones_f = singles.tile([128, 128], F32, tag="ones_f")
            nc.vector.memset(ones_f[:], 1.0)
            ones_r = singles.tile([128, 128], F32R, tag="ones_r")
            nc.vector.tensor_copy(ones_r[:], ones_f[:])
            ln20 = singles.tile([128, 1], F32, tag="ln20")
            nc.vector.memset(ln20[:], float(np.log(SMOOTH)))
            invC = singles.tile([128, 1], F32, tag="invC")
            nc.vector.memset(invC[:], 1.0 / C)
            rv_all = singles.tile([128, BPC], F32, tag="rv_all")

            ctx_t = [None] * (BPC + 5)   # plain ctx bf16 [128, 8jc, 1024d]
            ctxT_t = [None] * (BPC + 5)  # ctx^T bf16 [128, 8jd, 1024c]
            qT_t = [None] * (BPC + 5)    # q^T bf16 [128, 8jd, 128q]
            # per-batch in-flight state: (attn, sq) from head, chain tiles
            head_t = [None] * (BPC + 5)
            chain_t = [None] * (BPC + 5)

            h0, h1 = slice(0, 512), slice(512, 1024)

            def load_batch(b):
                ctx = ctxp.tile([128, 8, D], BF16, tag="ctx", name="ctx")
                nc.gpsimd.dma_start(out=ctx[:, 0:6, :], in_=c_in[b][:, 0:6, :])
                nc.sync.dma_start(out=ctx[:, 6:8, :], in_=c_in[b][:, 6:8, :])
                ctx_t[b] = ctx
                qT = qTp.tile([128, 8, Q], BF16, tag="qT", name="qT")
                nc.sync.dma_start(out=qT[:], in_=q_in[b])
                qT_t[b] = qT

            def load_ctxT(b):
                # ctx^T chunks 0..KT-1 straight from HBM (host-transposed)
                ctxT = ctxtp.tile([128, 8, C], BF16, tag="ctxT", name="ctxT")
                nc.scalar.dma_start(out=ctxT[:, 0:KT, :], in_=ct_in[b])
                if b < 3:
                    # prologue batches arrive fully transposed
                    nc.sync.dma_start(out=ctxT[:, KT:8, :], in_=ctp_in[b])
                ctxT_t[b] = ctxT

            def transpose_jd(b, jd, copy_eng):
                # PE-transpose ctx d-chunk jd via the PSUM ring, evict to SBUF
                ctx = ctx_t[b]
                tp = ps_tp.tile([128, 8, 128], BF16, tag="tp", name="tp")
                for jc in range(8):
                    nc.tensor.transpose(
                        tp[:, jc, :],
                        ctx[:, jc, jd * 128:(jd + 1) * 128], identb[:])
                src = tp[:].rearrange("p a b -> p (a b)")
                dst = ctxT_t[b][:, jd, :]
                if copy_eng == "act":
                    nc.scalar.copy(dst, src)
                else:
                    nc.vector.tensor_copy(dst, src)

            def bmm1(b):
                # attn^T (q, c) accumulated over 8 d-chunks -> a0/a1
                mark(f'i{b}_bmm1')
                a0 = ps_a.tile([128, 512], F32, tag="a0", name="a0")
                a1 = ps_a.tile([128, 512], F32, tag="a1", name="a1")
                qT = qT_t[b]
                ctxT = ctxT_t[b]
                for jd in range(8):
                    st, sp = jd == 0, jd == 7
                    nc.tensor.matmul(a0[:], qT[:, jd, :], ctxT[:, jd, 0:512],
                                     start=st, stop=sp)
                    nc.tensor.matmul(a1[:], qT[:, jd, :], ctxT[:, jd, 512:1024],
                                     start=st, stop=sp)
                ctxT_t[b] = None
                qT_t[b] = None
                return a0, a1

            def emit_head(b, a0, a1):
                # prelu (ACT, frees the a-banks) then sq (DVE)
                mark(f'i{b}_head')
                attn = work.tile([128, C], F32, tag="attn", name="attn")
                nc.scalar.activation(attn[:, h0], a0[:], ACTF.Prelu,
                                     bias=0.0, scale=1.0, alpha=0.1)
                nc.scalar.activation(attn[:, h1], a1[:], ACTF.Prelu,
                                     bias=0.0, scale=1.0, alpha=0.1)
                sq = w1.tile([128, C], F32R, tag="w1a", name="sq")
                nc.vector.tensor_mul(sq[:, h0], attn[:, h0], attn[:, h0])
                nc.vector.tensor_mul(sq[:, h1], attn[:, h1], attn[:, h1])
                head_t[b] = (attn, sq)

            def emit_ones(b):
                # l2 sums: ones-matmul broadcasts S to all q partitions
                mark(f'i{b}_ones')
                _, sq = head_t[b]
                s0 = ps_s.tile([128, 512], F32, tag="s0", name="s0")
                s1 = ps_s.tile([128, 512], F32, tag="s1", name="s1")
                nc.tensor.matmul(s0[:], ones_r[:], sq[:, h0], start=True, stop=True)
                nc.tensor.matmul(s1[:], ones_r[:], sq[:, h1], start=True, stop=True)
                head_t[b] = (head_t[b][0], s0, s1)

            def emit_ln(b):
                # 20/sqrt(S) = exp(-0.5*ln(S) + ln 20) on ACT
                mark(f'i{b}_ln')
                attn, s0, s1 = head_t[b]
                lnS = w1.tile([128, C], F32, tag="w1b", name="lnS")
                nc.scalar.activation(lnS[:, h0], s0[:], ACTF.Ln)
                nc.scalar.activation(lnS[:, h1], s1[:], ACTF.Ln)
                rn20 = w1.tile([128, C], F32, tag="w1c", name="rn20")
                nc.scalar.activation(rn20[:, h0], lnS[:, h0], ACTF.Exp,
                                     bias=ln20[:], scale=-0.5)
                nc.scalar.activation(rn20[:, h1], lnS[:, h1], ACTF.Exp,
                                     bias=ln20[:], scale=-0.5)
                head_t[b] = (attn, rn20)

            def emit_late(b):
                # u (Pool) -> exp+accum (ACT) -> thr -> focal (DVE) -> rinv
                mark(f'i{b}_late')
                attn, rn20 = head_t[b]
                u = w1.tile([128, C], F32, tag="w1d", name="u")
                nc.vector.tensor_mul(u[:, h0], attn[:, h0], rn20[:, h0])
                nc.gpsimd.tensor_mul(u[:, h1], attn[:, h1], rn20[:, h1])
                pu = work.tile([128, C], F32, tag="pu", name="pu")
                rs0 = stat.tile([128, 1], F32, tag="rs0", name="rs0")
                rs1 = stat.tile([128, 1], F32, tag="rs1", name="rs1")
                nc.scalar.activation(pu[:, h0], u[:, h0], ACTF.Exp,
                                     bias=0.0, scale=1.0, accum_out=rs0[:])
                nc.scalar.activation(pu[:, h1], u[:, h1], ACTF.Exp,
                                     bias=0.0, scale=1.0, accum_out=rs1[:])
                thr = stat.tile([128, 1], F32, tag="thr", name="thr")
                nc.vector.scalar_tensor_tensor(
                    out=thr[:], in0=rs0[:], scalar=rs1[:], in1=invC[:],
                    op0=ALU.add, op1=ALU.mult)
                mark(f'i{b}_focal')
                t = tpool.tile([128, C], BF16, tag="t", name="t")
                ts0 = stat.tile([128, 1], F32, tag="ts0", name="ts0")
                ts1 = stat.tile([128, 1], F32, tag="ts1", name="ts1")
                nc.vector.scalar_tensor_tensor(
                    out=t[:, h0], in0=pu[:, h0], scalar=thr[:], in1=pu[:, h0],
                    op0=ALU.is_gt, op1=ALU.mult, accum_out=ts0[:])
                nc.vector.scalar_tensor_tensor(
                    out=t[:, h1], in0=pu[:, h1], scalar=thr[:], in1=pu[:, h1],
                    op0=ALU.is_gt, op1=ALU.mult, accum_out=ts1[:])
                ts = stat.tile([128, 1], F32, tag="ts", name="ts")
                nc.vector.tensor_add(ts[:], ts0[:], ts1[:])
                nc.vector.reciprocal(rv_all[:, b:b + 1], ts[:])
                chain_t[b] = t
                head_t[b] = None

            def emit_tail_pe(b):
                # t^T transposes + bmm2 (all inputs finished last iteration)
                mark(f'i{b}_tT')
                t = chain_t[b]
                tT = tTp.tile([128, 8, Q], BF16, tag="tT", name="tT")
                tpf = ps_tp.tile([128, 8, 128], BF16, tag="tp", name="tpf")
                for jc in range(8):
                    nc.tensor.transpose(
                        tpf[:, jc, :],
                        t[:, jc * 128:(jc + 1) * 128], identb[:])
                    if jc == 3:
                        nc.vector.tensor_copy(
                            tT[:, 0:4, :].rearrange("p a b -> p (a b)"),
                            tpf[:, 0:4, :].rearrange("p a b -> p (a b)"))
                nc.vector.tensor_copy(
                    tT[:, 4:8, :].rearrange("p a b -> p (a b)"),
                    tpf[:, 4:8, :].rearrange("p a b -> p (a b)"))
                mark(f'i{b}_bmm2')
                ctx = ctx_t[b]
                w0 = ps_w.tile([128, 512], F32, tag="w0", name="w0")
                w2 = ps_w.tile([128, 512], F32, tag="w2", name="w2")
                for jc in range(8):
                    st, sp = jc == 0, jc == 7
                    nc.tensor.matmul(w0[:], tT[:, jc, :], ctx[:, jc, 0:512],
                                     start=st, stop=sp)
                    nc.tensor.matmul(w2[:], tT[:, jc, :], ctx[:, jc, 512:1024],
                                     start=st, stop=sp)
                ctx_t[b] = None
                return w0, w2

            def emit_wc(b, w0, w2):
                # evict bmm2 with the 1/sum(t) renorm folded in, store bf16
                mark(f'i{b}_wc')
                rv = rv_all[:, b:b + 1]
                wc = work.tile([128, D], BF16, tag="wc", name="wc")
                nc.scalar.activation(wc[:, h0], w0[:], ACTF.Copy,
                                     bias=0.0, scale=rv)
                _wc1 = nc.vector.tensor_scalar(wc[:, h1], w2[:], rv, None,
                                               ALU.mult)
                if b == BPC - 1:
                    wcd = wc_out[b].rearrange("q (g e) -> q g e", g=4)
                    wcs = wc[:].rearrange("q (g e) -> q g e", g=4)
                    nc.sync.dma_start(out=wcd[:, 0:2, :], in_=wcs[:, 0:2, :])
                    nc.gpsimd.dma_start(out=wcd[:, 2:3, :], in_=wcs[:, 2:3, :])
                    nc.scalar.dma_start(out=wcd[:, 3:4, :], in_=wcs[:, 3:4, :])
                else:
                    nc.scalar.dma_start(out=wc_out[b], in_=wc[:])

            # ---- prologue: fill the 3-phase pipeline.
            ctx0 = ctxp.tile([128, 8, D], BF16, tag="ctx", name="ctx")
            nc.gpsimd.dma_start(out=ctx0[:, 0:3, :], in_=c_in[0][:, 0:3, :])
            nc.sync.dma_start(out=ctx0[:, 3:6, :], in_=c_in[0][:, 3:6, :])
            nc.scalar.dma_start(out=ctx0[:, 6:8, :], in_=c_in[0][:, 6:8, :])
            ctx_t[0] = ctx0
            qT0 = qTp.tile([128, 8, Q], BF16, tag="qT", name="qT")
            nc.scalar.dma_start(out=qT0[:], in_=q_in[0])
            qT_t[0] = qT0
            load_ctxT(0)
            load_batch(1)
            load_ctxT(1)
            load_batch(2)
            load_ctxT(2)
            load_batch(3)
            a_pre = bmm1(0)
            emit_head(0, *a_pre)
            emit_ones(0)
            a_bank = {1: bmm1(1)}

            w_prev = None
            for b in range(BPC + 1):
                mark(f'iter{b}')
                if b + 4 < BPC:
                    load_batch(b + 4)
                if b + 3 < BPC:
                    load_ctxT(b + 3)
                if b >= 1:
                    w_prev = emit_tail_pe(b - 1)
                if b + 1 < BPC:
                    emit_head(b + 1, *a_bank.pop(b + 1))
                if b < BPC:
                    emit_ln(b)
                if b + 1 < BPC:
                    emit_ones(b + 1)
                if b + 3 < BPC:
                    mark(f'i{b}_tp')
                    transpose_jd(b + 3, KT + 0, "vec")
                    transpose_jd(b + 3, KT + 1, "vec")
                    transpose_jd(b + 3, KT + 2, "vec")
                    transpose_jd(b + 3, KT + 3, "vec")
                if b < BPC:
                    emit_late(b)
                if b + 2 < BPC:
                    a_bank[b + 2] = bmm1(b + 2)
                if b < BPC:
                    if b == BPC - 1:
                        nc.scalar.dma_start(out=t_out[b][:, 0:512],
                                            in_=chain_t[b][:, 0:512])
                        nc.sync.dma_start(out=t_out[b][:, 512:1024],
                                          in_=chain_t[b][:, 512:1024])
                    else:
                        nc.sync.dma_start(out=t_out[b], in_=chain_t[b][:])
                if b >= 1:
                    emit_wc(b - 1, *w_prev)
                    chain_t[b - 1] = None

            nc.sync.dma_start(out=rv_out[:, :], in_=rv_all[:])

    nc.compile()
    return nc


def kernel(query: np.ndarray, context: np.ndarray):
    query = np.ascontiguousarray(query, dtype=np.float32)
    context = np.ascontiguousarray(context, dtype=np.float32)
    assert query.shape == (NB, Q, D) and context.shape == (NB, C, D)

    if "nc" not in _CACHE:
        _CACHE["nc"] = _build()
    nc = _CACHE["nc"]

    bf16 = ml_dtypes.bfloat16
    # qT host prep: (B, Q, D) -> [b, p, jd, q] where d = jd*128 + p
    qT = np.ascontiguousarray(
        query.transpose(0, 2, 1).reshape(NB, 8, 128, Q).transpose(0, 2, 1, 3)
    ).astype(bf16)
    # context: (B, C, D) -> [b, p, jc, d] with c = jc*128 + p
    ctx_bf = np.ascontiguousarray(
        context.reshape(NB, 8, 128, D).transpose(0, 2, 1, 3)
    ).astype(bf16)
    # context^T chunks 0..KT-1: [b, p, jd, c] with d = jd*128 + p
    ctxT = np.ascontiguousarray(
        context[:, :, 0:KT * 128].transpose(0, 2, 1)
        .reshape(NB, KT, 128, C).transpose(0, 2, 1, 3)
    ).astype(bf16)
    # remaining chunks for each core's first 3 batches (prologue fill)
    pro_idx = np.concatenate([np.arange(k * BPC, k * BPC + 3)
                              for k in range(NCORES)])
    ctxT_pro = np.ascontiguousarray(
        context[pro_idx][:, :, KT * 128:].transpose(0, 2, 1)
        .reshape(3 * NCORES, 8 - KT, 128, C).transpose(0, 2, 1, 3)
    ).astype(bf16)

    in_maps = []
    for k in range(NCORES):
        sl = slice(k * BPC, (k + 1) * BPC)
        in_maps.append({"query": qT[sl], "context": ctx_bf[sl],
                        "contextT": ctxT[sl],
                        "contextTpro": ctxT_pro[3 * k:3 * k + 3]})

    trace = os.environ.get("KERNEL_TRACE", "0") == "1"
    res = run_bass_kernel_spmd(nc, in_maps, core_ids=list(range(NCORES)),
                               trace=trace)
    _CACHE["last_res"] = res

    # host renorm: re = t * rinv (broadcast over c), wc upcast
    re_parts = []
    wc_parts = []
    for r in res.results:
        t = np.asarray(r["t_out"]).astype(np.float32)        # [BPC, Q, C]
        rv = np.asarray(r["rv_out"]).astype(np.float32)      # [Q, BPC]
        wc = np.asarray(r["wcontext"]).astype(np.float32)    # [BPC, Q, D]
        re_parts.append(t * rv.T[:, :, None])
        wc_parts.append(wc)
    re_attn = np.concatenate(re_parts, axis=0)
    wcontext = np.concatenate(wc_parts, axis=0)
    return query, wcontext, re_attn
